# revision 91
# baseline (speedup 1.0000x reference)
"""DGMC (deep graph matching consensus) Bass kernel for 8 Trainium2 NeuronCores.

Problem (see reference):
  B=4 graph pairs, N=1024 nodes/graph, F_IN=128, F_HID=256, R=16, 2 steps,
  E=65536 random edges per graph side (edges span the whole 4096-node set).

  h_s = relu(((I+A_s) @ x_s) @ W1 + b1);  same for h_t
  S_hat = h_s @ h_t^T  per batch                      [B, N, N]
  S_0 = softmax(S_hat)
  2x: S = softmax(S_hat); r_t = S^T @ r_s
      o_s = relu(((I+A_s) @ r_s) @ W2 + b2); o_t likewise with r_t
      S_hat += mlp(o_s[s] - o_t[t]) pointwise over [B, N, N, R]
  S_L = softmax(S_hat)

Sharding: core c = 2*b + h handles batch b, s-half h (512 similarity rows)
AND only its own 512 t-columns of everything t-sided (adjacency columns,
agg_t, o_t).  The partner core's halves arrive via pair AllReduce
exchanges; recovering partner = sum - own keeps the program SPMD-uniform
(no parity-dependent addressing).  S_hat/pexp columns live in LOCAL order
(own half first) and assemble() unpermutes on the host; the r_t AllGather
un-swaps odd cores' halves with a static AP swap in the pair-sum.
Collectives per run: 1 vt-pair-AllReduce, 2 r_t 8-core AllGathers (fp16
payload), 2 otT pair-AllReduces (fp16).

Phase structure / overlap:
  - one merged chunk loop runs five software-pipelined matmul streams
    (s-hi, s-lo, o_s-preagg, t-hi, t-lo) with staggered trails matched to
    the arrival order of few large DMAs (HWDGE issue and the DMA engines
    serialize, so transfer count and order matter more than bytes)
  - adjacency matrices are exact small ints: stored fp8e4m3 (att/ats),
    streamed against bf16 hi/lo x operands
  - S_hat tiles for the own t-half run while the vt exchange is in
    flight; the partner half's W1 runs locally afterwards
  - softmax is pipelined two (sc, tb) groups behind the S_hat/D folds
    (split half-row maxes; exp/reciprocal/partial-r_t trail) so no engine
    stream ever blocks ht production
  - the o_s->A' chain and streams of scrap warm-keeper matmuls fill the
    collective gaps so the PE p-state never drops

Precision strategy: values feeding softmax need ~1e-5 relative accuracy on
the S_hat scale (|S_hat| reaches ~2.5e3): x and r_t are exact bf16 hi/lo
pairs, h/S_hat matmuls run true fp32.  The MLP D-phase tolerates ~2^-11
(measured: fp16 H leaves S_L rel err ~1e-3, gate is 2e-2):
  A'[s,k] = (o_s @ Wm1' + bm1')[s,k]   (fp32, bounced to a [(s8,k), j]
            per-partition bias layout through DRAM with contiguous 256B
            runs via an (e j)-permuted matmul free order)
  negrep[(s8,k), t] = -(o_t @ Wm1')[t,k]  (fp16, via an M-tiled weights
            matmul that replicates the 16 k-rows over 8 s8 groups)
  H(j) = relu(negrep + A'bias[:, j])  -- one DVE(4x)/ACT/GPSIMD
            tensor-scalar op per (j, t-half) tile; no PE first-matmul
  psum2[s, t] += Z_j^T @ H_j  -- k-reduction on the PE via the shifted
            sign matrix zbig (fp16, +-1 exact); DVE folds psum2 into
            S_hat (gpsimd cannot read PSUM).  Wm1' = Wm1 * |Wm2|; bm2 is
            dropped (softmax is shift-invariant); outputs stored bf16.
"""

import numpy as np
import ml_dtypes

import concourse.bass as bass
import concourse.bacc as bacc
import concourse.tile as tile
from concourse import mybir
from concourse import bass_utils

F32 = mybir.dt.float32
F32R = mybir.dt.float32  # fp32r disabled: hangs on HW
BF16 = mybir.dt.bfloat16
F16 = mybir.dt.float16
F8 = mybir.dt.float8e4

B, N, F_IN, F_HID, R, NUM_STEPS, E = 4, 1024, 128, 256, 16, 2, 65536
NNODE = B * N
NCHUNK = NNODE // 128   # 32 source chunks
NCORES = 8
SH = N // 2             # 512 s rows per core
NSC = SH // 128         # 4 s-chunks per core
NJ = SH // 8            # 64 8-row j blocks per core
AF = mybir.ActivationFunctionType
ALU = mybir.AluOpType


def _bf_split(x):
    hi = np.asarray(x, dtype=np.float32).astype(ml_dtypes.bfloat16)
    lo = (np.asarray(x, dtype=np.float32) - hi.astype(np.float32)).astype(
        ml_dtypes.bfloat16)
    return hi, lo


def _chunk_major(x):
    """[4096, W] -> [128, 32*W] with col (c*W + w) = x[c*128 + p, w]."""
    w = x.shape[1]
    return np.ascontiguousarray(
        x.reshape(NCHUNK, 128, w).transpose(1, 0, 2).reshape(128, NCHUNK * w))


def _adjT_plus_I(edge_index):
    """(A^T + I) as float32 (exact small ints); A[dst, src] = edge count."""
    src = np.asarray(edge_index[0], dtype=np.int64)
    dst = np.asarray(edge_index[1], dtype=np.int64)
    flat = src * NNODE + dst
    cnt = np.bincount(flat, minlength=NNODE * NNODE).astype(np.float32)
    at = cnt.reshape(NNODE, NNODE)
    at[np.arange(NNODE), np.arange(NNODE)] += 1.0
    return at


_BUILD_CACHE = {}


def _build(kp, bm2val, mock_cc=False):
    key = (kp, float(bm2val), mock_cc)
    if key in _BUILD_CACHE:
        return _BUILD_CACHE[key]

    nc = bacc.Bacc("TRN2", target_bir_lowering=False, debug=False,
                   num_devices=NCORES)

    # ---- DRAM I/O (per-core data) ----
    # att holds only this core's OWN 512 t-columns; the partner's half of
    # everything t-sided arrives via pair AllReduce exchanges (partner =
    # sum - own keeps the program SPMD-uniform).  S_hat/D-phase columns are
    # in LOCAL order (own half first); assemble() unpermutes on the host.
    d_att = nc.dram_tensor("att", [128, NCHUNK * 512], F8, kind="ExternalInput")
    d_ats = nc.dram_tensor("ats", [NCHUNK, 128, SH], F8, kind="ExternalInput")
    d_xshi = nc.dram_tensor("xshi", [128, NCHUNK * F_IN], BF16, kind="ExternalInput")
    d_xslo = nc.dram_tensor("xslo", [128, NCHUNK * F_IN], BF16, kind="ExternalInput")
    d_xthi = nc.dram_tensor("xthi", [128, NCHUNK * F_IN], BF16, kind="ExternalInput")
    d_xtlo = nc.dram_tensor("xtlo", [128, NCHUNK * F_IN], BF16, kind="ExternalInput")
    # packed o_s lhsT: per chunk, 112 cols = step0-hi@0, step0-lo@32,
    # step1-hi@64, step1-lo@96 (zeros between) -> one M=112 matmul per chunk
    d_rsnp = nc.dram_tensor("rsnp", [128, NCHUNK * 112], BF16,
                            kind="ExternalInput")
    d_rsown = nc.dram_tensor("rsown", [128, NUM_STEPS * NSC * R], F32,
                             kind="ExternalInput")
    d_w1 = nc.dram_tensor("w1", [F_IN, F_HID], F32, kind="ExternalInput")
    d_b1c = nc.dram_tensor("b1c", [128, 2], F32, kind="ExternalInput")
    d_w2 = nc.dram_tensor("w2", [R, R], F32, kind="ExternalInput")
    d_w2st = nc.dram_tensor("w2st", [48, R], F32, kind="ExternalInput")
    d_b2c = nc.dram_tensor("b2c", [R, 1], F32, kind="ExternalInput")
    d_wm1pA = nc.dram_tensor("wm1pA", [R, R], F32, kind="ExternalInput")
    d_wm1pPt = nc.dram_tensor("wm1pPt", [R, 128], F16, kind="ExternalInput")
    d_bm1pc = nc.dram_tensor("bm1pc", [R, 1], F32, kind="ExternalInput")
    d_zbig = nc.dram_tensor("zbig", [128, 248], F16, kind="ExternalInput")
    # A' bias bounce: [(s8,k), j] layout per step
    d_ab = nc.dram_tensor("ab", [NUM_STEPS, 128, NJ], F32)

    d_s0 = nc.dram_tensor("s0o", [SH, N], BF16, kind="ExternalOutput")
    d_sl = nc.dram_tensor("slo", [SH, N], BF16, kind="ExternalOutput")

    # collective buffers (per step); AllGather concatenates along dim 0
    d_ccin = [nc.dram_tensor(f"ccin{i}", [128, 128], F16) for i in range(NUM_STEPS)]
    d_ccout = [nc.dram_tensor(f"ccout{i}", [NCORES * 128, 128], F16,
                              addr_space="Shared") for i in range(NUM_STEPS)]
    # pair-exchange buffers (AllReduce over core pairs)
    PAIRS = [[2 * b, 2 * b + 1] for b in range(B)]
    d_hxin = nc.dram_tensor("hxin", [128, 512], F32)
    d_hxout = nc.dram_tensor("hxout", [128, 512], F32)
    d_oxin = [nc.dram_tensor(f"oxin{i}", [R, 512], F16) for i in range(NUM_STEPS)]
    d_oxout = [nc.dram_tensor(f"oxout{i}", [R, 512], F16)
               for i in range(NUM_STEPS)]

    with tile.TileContext(nc) as tc:
        # ---------- resident tiles ----------
        with tc.tile_pool(name="res", bufs=1) as res:
            att = res.tile([128, NCHUNK * 512], F8)
            atsr = res.tile([128, NCHUNK * SH], F8)
            zbig = res.tile([128, 248], F16)
            rsnp = res.tile([128, NCHUNK * 112], BF16)
            rt48 = res.tile([128, NCHUNK * 48], BF16)
            # split memset: a small fast region unblocks the warm matmuls
            # ~1.8us earlier than one full-tile memset would
            nc.gpsimd.memset(rt48[:, 0:256], 0.0)
            warm_ctx = tc.tile_pool(name="warm", bufs=1, space="PSUM")
            warmp = warm_ctx.__enter__()
            wscr = warmp.tile([16, 256], F32)
            for _ in range(16):
                nc.tensor.matmul(wscr[:], rt48[:, 0:16], rt48[:, 0:256],
                                 start=True, stop=True)
            warm_ctx.__exit__(None, None, None)
            nc.gpsimd.memset(rt48[:, 256:], 0.0)
            rsown = res.tile([128, NUM_STEPS * NSC * R], F32)
            w1 = res.tile([F_IN, F_HID], F32)
            b1c = res.tile([128, 2], F32)
            w2 = res.tile([R, R], F32)
            w2st = res.tile([48, R], F32)
            w1r = res.tile([F_IN, F_HID], F32R)
            w2str = res.tile([48, R], F32R)
            wm1pAr = res.tile([R, R], F32R)
            b2c = res.tile([R, 1], F32)
            wm1pA = res.tile([R, R], F32)
            wm1pPt = res.tile([R, 128], F16)
            bm1pc = res.tile([R, 1], F32)
            abias = res.tile([128, NUM_STEPS * NJ], F32)
            negrep = res.tile([128, N], F16)

            shat = res.tile([128, NSC * N], F32)      # [128, (sc, t)]

            smp = res  # softmax tiles live in the resident pool
            out_pool_ctx = tc.tile_pool(name="outp", bufs=2)
            out_pool = out_pool_ctx.__enter__()
            pexp = smp.tile([128, NSC * N], F32)   # exp(shat - max)
            rzt = [dict(n0=smp.tile([128, 1], F32, tag=f"n0{sc}", name=f"n0{sc}"),
                        n1=smp.tile([128, 1], F32, tag=f"n1{sc}", name=f"n1{sc}"),
                        nm=smp.tile([128, 1], F32, tag=f"nm{sc}", name=f"nm{sc}"),
                        z=smp.tile([128, 1], F32, tag=f"z{sc}", name=f"z{sc}"),
                        rz=smp.tile([128, 1], F32, tag=f"rz{sc}", name=f"rz{sc}"))
                   for sc in range(NSC)]
            zcol = smp.tile([128, 1], F32, tag="zcol", name="zcol")
            nc.gpsimd.memset(zcol[:], 0.0)
            emit_out = []

            def flush_out():
                while emit_out:
                    sc, ssl, rz, out_dram = emit_out.pop(0)
                    so = out_pool.tile([128, N], BF16, tag="so", name="so")
                    nc.gpsimd.tensor_scalar_mul(so[:], pexp[:, ssl],
                                                rz[:, 0:1])
                    nc.sync.dma_start(
                        out_dram.ap()[sc * 128:(sc + 1) * 128, :], so[:])

            def stage_max(sc, tb, src):
                """stage psum half into shat (tb0 DVE / tb1 ACT), row max
                on DVE -- the verified baseline's two-op form."""
                ssl = slice(sc * N + tb * 512, sc * N + (tb + 1) * 512)
                if tb == 0:
                    nc.vector.tensor_copy(shat[:, ssl], src[:])
                else:
                    nc.scalar.copy(shat[:, ssl], src[:])
                nc.vector.reduce_max(rzt[sc]["n0" if tb == 0 else "n1"][:],
                                     shat[:, ssl], negate=True,
                                     axis=mybir.AxisListType.X)

            def fold_max(sc, tb, src):
                """fold psum into shat, then row max (baseline form)."""
                ssl = slice(sc * N + tb * 512, sc * N + (tb + 1) * 512)
                nc.vector.tensor_add(shat[:, ssl], shat[:, ssl], src[:])
                nc.vector.reduce_max(rzt[sc]["n0" if tb == 0 else "n1"][:],
                                     shat[:, ssl], negate=True,
                                     axis=mybir.AxisListType.X)

            def sm_exp(sc):
                """combine (negated) half-maxes, exp row block sc."""
                ssl = slice(sc * N, (sc + 1) * N)
                nm = rzt[sc]["nm"]
                nc.vector.tensor_tensor(nm[:], rzt[sc]["n0"][:],
                                        rzt[sc]["n1"][:], ALU.min)
                nc.scalar.activation(pexp[:, ssl], shat[:, ssl], AF.Exp,
                                     bias=nm[:, 0:1],
                                     accum_out=rzt[sc]["z"][:, 0:1])

            rsps = [res.tile([128, NSC * R], F32, tag=f"rsp{i}",
                             name=f"rsp{i}") for i in range(NUM_STEPS)]

            def sm_fin(i, sc, prt, out_dram, defer=True):
                """reciprocal, then either the partial-r_t products for the
                next step (prt) or the normalized output write."""
                rz = rzt[sc]["rz"]
                nc.vector.reciprocal(rz[:], rzt[sc]["z"][:])
                ssl = slice(sc * N, (sc + 1) * N)
                if prt is not None:
                    rsp = rsps[i]
                    nc.vector.tensor_scalar_mul(
                        rsp[:, sc * R:(sc + 1) * R],
                        rsown[:, i * NSC * R + sc * R:
                              i * NSC * R + (sc + 1) * R],
                        rz[:, 0:1])
                    for tcn in range(8):
                        # two psum accumulation groups (tcn 0-3 / 4-7): the
                        # first half closes a few matmuls early so its ccs
                        # copy + ccin DMA overlap the trail's last ops
                        nc.tensor.matmul(
                            prt[:, tcn * R:(tcn + 1) * R],
                            pexp[:, sc * N + tcn * 128:
                                 sc * N + (tcn + 1) * 128],
                            rsp[:, sc * R:(sc + 1) * R],
                            start=(sc == 0 and tcn % 4 == 0),
                            stop=(sc == NSC - 1 and tcn % 4 == 3))
                if out_dram is not None:
                    if defer:
                        emit_out.append((sc, ssl, rz, out_dram))
                    else:
                        so = out_pool.tile([128, N], BF16, tag="so", name="so")
                        if sc == NSC - 1:
                            nc.vector.tensor_scalar_mul(so[:], pexp[:, ssl],
                                                        rz[:, 0:1])
                        else:
                            nc.gpsimd.tensor_scalar_mul(so[:], pexp[:, ssl],
                                                        rz[:, 0:1])
                        nc.sync.dma_start(
                            out_dram.ap()[sc * 128:(sc + 1) * 128, :], so[:])

            # partial r_t psum accumulators (one per step) outlive the
            # phase pools: S_hat-phase softmaxes feed prt0, D-loop(0) feeds
            # prt1
            pPr_ctx = tc.tile_pool(name="pPr", bufs=2, space="PSUM")
            pPr = pPr_ctx.__enter__()
            prts = [pPr.tile([128, 128], F32, tag="prt", name=f"prt{i}")
                    for i in range(NUM_STEPS)]
            # o_s pre-aggregate survives phase A: its W2/Wm1' chain runs
            # inside the step-0 collective gap to keep the PE warm there
            pAo_ctx = tc.tile_pool(name="pAo", bufs=1, space="PSUM")
            pAo = pAo_ctx.__enter__()
            aggo = pAo.tile([112, SH], F32)

            # ---------- phase A: psi_1 aggregates + o_s chains ----------
            with tc.tile_pool(name="xA", bufs=1) as xA, \
                 tc.tile_pool(name="wA", bufs=1) as wA:
                pA1_ctx = tc.tile_pool(name="pA1", bufs=1, space="PSUM")
                pA = pA1_ctx.__enter__()
                xshi = xA.tile([128, NCHUNK * F_IN], BF16)
                xslo = xA.tile([128, NCHUNK * F_IN], BF16)
                xthi = xA.tile([128, NCHUNK * F_IN], BF16)
                xtlo = xA.tile([128, NCHUNK * F_IN], BF16)
                # load x_s-hi/lo in 4 column slices so the first agg
                # matmuls only wait on an eighth of the bytes; the lo and
                # aggo matmuls trail the hi stream by 2/4 chunks so their
                # operand slices arrive in time
                XSL = NCHUNK * F_IN // 4
                ATL = NCHUNK * 512 // 4
                # first att piece is an eighth so the first t-hi matmul only
                # waits on 256KB; the rest of quarter 0 follows as pop #1
                nc.sync.dma_start(xthi[:, 0:XSL], d_xthi.ap()[:, 0:XSL])
                nc.sync.dma_start(att[:, 0:ATL // 2], d_att.ap()[:, 0:ATL // 2])

                def _sl_load(t_, d_, q, w):
                    qs = slice(q * w, (q + 1) * w)
                    return lambda: nc.sync.dma_start(t_[:, qs], d_.ap()[:, qs])

                def _full_load(t_, d_):
                    return lambda: nc.sync.dma_start(t_[:], d_.ap())

                def _ats_load(g):
                    return lambda: nc.sync.dma_start(
                        atsr[:, g * 4 * SH:(g + 1) * 4 * SH].rearrange(
                            "p (g x) -> p g x", g=4),
                        d_ats.ap()[4 * g:4 * (g + 1)].rearrange(
                            "g p x -> p g x"))

                # HWDGE serializes DMA issue and the DMA engines serialize
                # transfers, so phase A uses few, large transfers ordered by
                # their consumption deadline in the merged chunk loop
                # (t streams lead now, so xt/att quarters have the earliest
                # deadlines; s/aggo trail and their operands arrive later)
                deferred_loads = [
                    lambda: nc.sync.dma_start(att[:, ATL // 2:ATL],
                                              d_att.ap()[:, ATL // 2:ATL]),
                    _sl_load(xtlo, d_xtlo, 0, XSL),
                    _ats_load(0),
                    _sl_load(xshi, d_xshi, 0, XSL),
                    _sl_load(att, d_att, 1, ATL),
                    _sl_load(xslo, d_xslo, 0, XSL),
                    _sl_load(xthi, d_xthi, 1, XSL),
                    _ats_load(1),
                    _sl_load(xtlo, d_xtlo, 1, XSL),
                    _sl_load(xshi, d_xshi, 1, XSL),
                    _sl_load(att, d_att, 2, ATL),
                    _sl_load(xslo, d_xslo, 1, XSL),
                    _sl_load(xthi, d_xthi, 2, XSL),
                    _ats_load(2),
                    _ats_load(3),
                    _full_load(rsnp, d_rsnp),
                    _sl_load(xtlo, d_xtlo, 2, XSL),
                    _sl_load(xshi, d_xshi, 2, XSL),
                    _sl_load(att, d_att, 3, ATL),
                    _sl_load(xslo, d_xslo, 2, XSL),
                    _sl_load(xthi, d_xthi, 3, XSL),
                    _sl_load(xtlo, d_xtlo, 3, XSL),
                    _ats_load(4), _ats_load(5),
                    _sl_load(xshi, d_xshi, 3, XSL),
                    _sl_load(xslo, d_xslo, 3, XSL),
                    _ats_load(6), _ats_load(7),
                    _full_load(rsown, d_rsown), _full_load(w1, d_w1),
                    _full_load(b1c, d_b1c), _full_load(w2, d_w2),
                    _full_load(w2st, d_w2st),
                    _full_load(b2c, d_b2c), _full_load(wm1pA, d_wm1pA),
                    _full_load(wm1pPt, d_wm1pPt), _full_load(bm1pc, d_bm1pc),
                    _full_load(zbig, d_zbig),
                ]
                agg_s = pA.tile([128, SH], F32)
                agg_t = pA.tile([128, 512], F32)
                # one merged loop: five matmul streams (t-hi, t-lo, s-hi,
                # s-lo, aggo) with staggered trails so every stream's
                # operands have landed by the time the in-order PE reaches
                # it.  The t streams LEAD so agg_t closes ~8 iterations
                # before the loop ends: the h_t pair-exchange's 3-hop
                # DMA-chain latency then hides completely under the s/aggo
                # trail, and S_hat can start on both halves immediately.
                # aggo trails far behind: the merged loop's s/aggo tail
                # after agg_t closes covers the full h_t pair-exchange
                # 3-hop DMA latency, so hsub is ready when S_hat needs it
                LO_T, HI_S, LO_S, AGO = 2, 4, 6, 20

                def _ats(c):
                    return atsr[:, c * SH:(c + 1) * SH]

                for c in range(NCHUNK + AGO):
                    if c < NCHUNK:
                        if deferred_loads:
                            deferred_loads.pop(0)()
                        nc.tensor.matmul(agg_t[:],
                                         xthi[:, c * F_IN:(c + 1) * F_IN],
                                         att[:, c * 512:(c + 1) * 512],
                                         start=(c == 0), stop=False)
                    if LO_T <= c < NCHUNK + LO_T:
                        cc = c - LO_T
                        nc.tensor.matmul(agg_t[:],
                                         xtlo[:, cc * F_IN:(cc + 1) * F_IN],
                                         att[:, cc * 512:(cc + 1) * 512],
                                         start=False, stop=(cc == NCHUNK - 1))
                    if HI_S <= c < NCHUNK + HI_S:
                        cc = c - HI_S
                        nc.tensor.matmul(agg_s[:],
                                         xshi[:, cc * F_IN:(cc + 1) * F_IN],
                                         _ats(cc), start=(cc == 0), stop=False)
                    if LO_S <= c < NCHUNK + LO_S:
                        cc = c - LO_S
                        nc.tensor.matmul(agg_s[:],
                                         xslo[:, cc * F_IN:(cc + 1) * F_IN],
                                         _ats(cc), start=False,
                                         stop=(cc == NCHUNK - 1))
                    if AGO <= c:
                        cc = c - AGO
                        nc.tensor.matmul(aggo[:],
                                         rsnp[:, cc * 112:(cc + 1) * 112],
                                         _ats(cc), start=(cc == 0),
                                         stop=(cc == NCHUNK - 1))
                while deferred_loads:
                    deferred_loads.pop(0)()
                # fp32r weight copies: walrus requires fp32r matmul inputs
                # to be rounded by their producing instruction, so the
                # DMA-loaded fp32 weights bounce through one engine copy
                nc.vector.tensor_copy(w1r[:], w1[:])
                nc.gpsimd.tensor_copy(w2str[:], w2st[:])
                nc.gpsimd.tensor_copy(wm1pAr[:], wm1pA[:])

                # stage the aggregate psums to SBUF (frees the A1 banks);
                # the own-half pre-W1 aggregate vt goes straight through the
                # pair AllReduce (partner = sum - own) while W1/relu/S_hat
                # own-half work covers the exchange latency
                h_sT = xA.tile([128, 2 * SH], F32R)   # [128, (fc, s)]
                h_own = xA.tile([128, 2 * 512], F32R)  # [128, (fc, own t)]
                hsub = xA.tile([128, 2 * 512], F32R)  # partner half h_t
                vs = wA.tile([128, SH], F32R)
                vt = wA.tile([128, 512], F32R)
                nc.scalar.copy(vt[:], agg_t[:])
                nc.sync.dma_start(d_hxin.ap(), vt[:].bitcast(F32))
                if mock_cc:
                    nc.sync.dma_start(d_hxout.ap(), d_hxin.ap())
                else:
                    nc.gpsimd.collective_compute(
                        "AllReduce", mybir.AluOpType.add,
                        replica_groups=PAIRS,
                        ins=[d_hxin.ap()], outs=[d_hxout.ap()])
                vtsum = wA.tile([128, 512], F32)
                nc.sync.dma_start(vtsum[:], d_hxout.ap())
                nc.scalar.copy(vs[:], agg_s[:])

                pA1_ctx.__exit__(None, None, None)
                pA2_ctx = tc.tile_pool(name="pA2", bufs=1, space="PSUM")
                pA = pA2_ctx.__enter__()

                for fc in range(2):
                    w1sl = w1r[:, fc * 128:(fc + 1) * 128]
                    ph2 = pA.tile([128, 512], F32, tag="ph1")
                    nc.tensor.matmul(ph2[:], w1sl, vt[:],
                                     start=True, stop=True)
                    # h_own/hsub relus on DVE: ACT alone serializing all six
                    # W1 relus plus the staging copies gated every S_hat tile
                    nc.vector.tensor_scalar(h_own[:, fc * 512:(fc + 1) * 512],
                                            ph2[:], b1c[:, fc:fc + 1],
                                            0.0, ALU.add, ALU.max)
                for fc in range(2):
                    w1sl = w1r[:, fc * 128:(fc + 1) * 128]
                    ph = pA.tile([128, 512], F32, tag="ph0")
                    nc.tensor.matmul(ph[:], w1sl, vs[:],
                                     start=True, stop=True)
                    nc.scalar.activation(h_sT[:, fc * SH:(fc + 1) * SH], ph[:],
                                         AF.Relu, bias=b1c[:, fc:fc + 1])
                vtP = wA.tile([128, 512], F32R)
                # NOTE: must stay OFF the Pool engine: gpsimd also issues
                # the collective_compute instructions, and a Pool op that
                # waits on a collective output can deadlock the in-order
                # Pool queue on real hardware (the mock build has no CC
                # instructions, so TimelineSim cannot see this)
                nc.vector.tensor_sub(vtP[:], vtsum[:], vt[:].bitcast(F32))

                # the exchange finished during the merged loop's s/aggo
                # trail, so the partner-half h_t is ready up front and the
                # S_hat tiles run sc-outer: each sc's softmax fires as soon
                # as its two halves land, shortening the trail by a group
                for fc in range(2):
                    w1sl = w1r[:, fc * 128:(fc + 1) * 128]
                    ph3 = pA.tile([128, 512], F32, tag="ph1")
                    nc.tensor.matmul(ph3[:], w1sl, vtP[:],
                                     start=True, stop=True)
                    nc.vector.tensor_scalar(
                        hsub[:, fc * 512:(fc + 1) * 512], ph3[:],
                        b1c[:, fc:fc + 1], 0.0, ALU.add, ALU.max)
                for sc in range(NSC):
                    for tb in range(2):
                        # rotate over four PSUM banks (the ph banks are dead
                        # after the W1 relus) so sc_k's matmuls never wait
                        # on sc_{k-1}'s staging copy
                        ps = pA.tile([128, 512], F32,
                                     tag=("pS" if sc % 2 == 0 else "ph")
                                     + str(tb))
                        hT = h_own if tb == 0 else hsub
                        for fc in range(2):
                            nc.tensor.matmul(
                                ps[:],
                                h_sT[:, fc * SH + sc * 128:
                                     fc * SH + (sc + 1) * 128],
                                hT[:, fc * 512:(fc + 1) * 512],
                                start=(fc == 0), stop=(fc == 1))
                        stage_max(sc, tb, ps)
                    sm_exp(sc)
                    if sc >= 1:
                        sm_fin(0, sc - 1, prts[0], d_s0)
                sm_fin(0, NSC - 1, prts[0], d_s0)
                pA2_ctx.__exit__(None, None, None)

            # ---------- steps ----------

            with tc.tile_pool(name="step", bufs=1) as stp, \
                 tc.tile_pool(name="pS", bufs=1, space="PSUM") as pSm, \
                 tc.tile_pool(name="pR", bufs=1, space="PSUM") as pR, \
                 tc.tile_pool(name="pD2", bufs=2, space="PSUM") as pD2, \
                 tc.tile_pool(name="hD", bufs=8) as hD:
                for i in range(NUM_STEPS):
                    # partial r_t products were accumulated as the softmaxes
                    # landed (S_hat loop for step 0, D-loop tail for step 1)
                    ccs = stp.tile([128, 128], F16, tag="ccs")
                    # DVE: the ACT queue is still draining trail exps here;
                    # the first prt column group closes a few matmuls early
                    # so this copy starts sooner
                    nc.vector.tensor_copy(ccs[:, 0:64], prts[i][:, 0:64])
                    nc.vector.tensor_copy(ccs[:, 64:128], prts[i][:, 64:128])
                    nc.sync.dma_start(d_ccin[i].ap(), ccs[:])
                    if mock_cc:
                        # stand-in for the AllGather so TimelineSim (single
                        # core, no collectives) can cost the kernel; one
                        # broadcast DMA moving the same 8x64KB the real
                        # collective delivers
                        nc.sync.dma_start(
                            d_ccout[i].ap().rearrange("(c p) x -> c p x",
                                                      c=NCORES),
                            d_ccin[i].ap().unsqueeze(0).broadcast_to(
                                [NCORES, 128, 128]))
                    else:
                        nc.gpsimd.collective_compute(
                            "AllGather", mybir.AluOpType.bypass,
                            replica_groups=[list(range(NCORES))],
                            ins=[d_ccin[i].ap()], outs=[d_ccout[i].ap()])
                    gath = stp.tile([128, NCORES * 128], F16, tag="gath")
                    nc.sync.dma_start(
                        gath[:].rearrange("p (c x) -> p c x", c=NCORES),
                        d_ccout[i].ap().rearrange("(c p) x -> p c x", c=NCORES))
                    if i == 0:
                        # o_s chains for both steps fill the collective-0
                        # gap with PE work: A' to the [(s8,k), j] bias
                        # layout, bounced through DRAM (contiguous 256B
                        # runs thanks to the (e j)-permuted free order of
                        # the pAT matmul rhs)
                        ab_loads = []
                        for ii in range(NUM_STEPS):
                            # stacked W2 contracts hi+lo pre-aggregate rows
                            # in one matmul (rows 16:32 of aggo are zero)
                            agc = stp.tile([48, SH], F32R, tag="agh")
                            nc.scalar.copy(agc[:], aggo[64 * ii:64 * ii + 48, :])
                            pz = pSm.tile([16, SH], F32, tag="pzt")
                            nc.tensor.matmul(pz[:], w2str[:], agc[:],
                                             start=True, stop=True)
                            osT = stp.tile([16, SH], F32R, tag="osT")
                            nc.scalar.activation(osT[:], pz[:], AF.Relu,
                                                 bias=b2c[:, 0:1])
                            paT = pSm.tile([16, SH], F32, tag="pzt")
                            nc.tensor.matmul(
                                paT[:], wm1pAr[:],
                                osT[:].rearrange("r (j e) -> r e j", e=8),
                                start=True, stop=True)
                            aTp = stp.tile([16, SH], F32, tag="aTp")
                            nc.vector.tensor_scalar_add(aTp[:], paT[:],
                                                        bm1pc[:, 0:1])
                            nc.sync.dma_start(
                                d_ab.ap()[ii].rearrange("(e k) j -> k e j",
                                                        k=16),
                                aTp[:])
                            # abias read-backs are deferred below the scrap
                            # emission: their HWDGE descriptor slots must not
                            # delay the r_t collective's hops
                            ab_loads.append(ii)
                        # scrap keeps the PE warm over the collective gap;
                        # tag "pago" (not "pzt") so it does not serialize
                        # against the o_s chains' psum bank ping-pong
                        scr0 = pSm.tile([16, 512], F32, tag="pago")
                        for _ in range(74):
                            nc.tensor.matmul(scr0[:], rsnp[:, 0:16],
                                             att[:, 0:512],
                                             start=True, stop=True)
                        for ii in ab_loads:
                            nc.sync.dma_start(abias[:, ii * NJ:(ii + 1) * NJ],
                                              d_ab.ap()[ii])
                    # pair-sum the gathered partials into global-order
                    # r_t.  Odd cores' pexp columns are locally permuted
                    # (own half first), so odd slots contribute their x
                    # halves swapped -- a static AP swap on the second add.
                    rt = stp.tile([128, 512], F32, tag="rt")
                    g4 = gath[:].rearrange("p (b h x) -> p b h x", b=4, h=2)
                    rtb = rt[:].rearrange("p (b x) -> p b x", b=4)
                    r4 = rt48[:].rearrange("p (b tc w) -> p b tc w",
                                           tc=8, w=48)
                    rv4 = rt[:].rearrange("p (b tc r) -> p b tc r",
                                          tc=8, r=R)
                    rthi32 = stp.tile([128, 512], F32, tag="rthi32")
                    rh4 = rthi32[:].rearrange("p (b tc r) -> p b tc r",
                                              tc=8, r=R)
                    # process the two x-halves of r_t independently so the
                    # first half's o_t aggregation matmuls can start after
                    # half the prep latency (the pago chunk order below
                    # visits the tc<4 chunks of every batch first)
                    # both halves on DVE: downstream of the r_t AllGather,
                    # so they must stay off the Pool/gpsimd queue (see the
                    # vtP note above)
                    for xh in range(2):
                        eng = nc.vector
                        xs = slice(xh * 64, (xh + 1) * 64)
                        xo = slice((1 - xh) * 64, (2 - xh) * 64)
                        tcs = slice(xh * 4, (xh + 1) * 4)
                        eng.tensor_add(rtb[:, :, xs],
                                       g4[:, :, 0, xs], g4[:, :, 1, xo])
                        eng.tensor_copy(r4[:, :, tcs, 0:16],
                                        rv4[:, :, tcs, :])
                        eng.tensor_copy(rh4[:, :, tcs, :],
                                        r4[:, :, tcs, 0:16])
                        eng.tensor_sub(r4[:, :, tcs, 32:48],
                                       rv4[:, :, tcs, :],
                                       rh4[:, :, tcs, :])

                    # o_t chain for the OWN t-half only: psi_2 aggregate,
                    # W2, relu; the partner's otT comes from the pair
                    # AllReduce (partner = sum - own); then the M-tiled
                    # Wm1' matmul replicates -P' over the 8 s8 groups for
                    # both local halves
                    pago = pSm.tile([48, 512], F32, tag="pago")
                    pago_order = [b * 8 + tc for xh in range(2)
                                  for b in range(4)
                                  for tc in range(xh * 4, (xh + 1) * 4)]
                    if i > 0:
                        # keep the PE p-state warm across the collective
                        # gap: a stream of scrap matmuls with no data deps
                        scr = pSm.tile([16, 512], F32, tag="pzt")
                        for _ in range(72):
                            nc.tensor.matmul(scr[:], rsnp[:, 0:16],
                                             att[:, 0:512],
                                             start=True, stop=True)
                    for ci, c in enumerate(pago_order):
                        nc.tensor.matmul(pago[:],
                                         rt48[:, c * 48:(c + 1) * 48],
                                         att[:, c * 512:(c + 1) * 512],
                                         start=(ci == 0),
                                         stop=(ci == NCHUNK - 1))
                    pcp = stp.tile([48, 512], F32R, tag="plo")
                    nc.scalar.copy(pcp[:], pago[0:48, :])
                    pzt = pSm.tile([16, 512], F32, tag="pzt")
                    nc.tensor.matmul(pzt[:], w2str[:], pcp[:], start=True,
                                     stop=True)
                    otT = stp.tile([16, 512], F16, tag="otT")
                    nc.scalar.activation(otT[:], pzt[:], AF.Relu,
                                         bias=b2c[:, 0:1])
                    nc.sync.dma_start(d_oxin[i].ap(), otT[:])
                    # bridge the pair-exchange latency with scrap so the PE
                    # p-state survives until the prep matmuls
                    scrx = pSm.tile([16, 512], F32, tag="pago")
                    for _ in range(12):
                        nc.tensor.matmul(scrx[:], rsnp[:, 0:16],
                                         att[:, 0:512], start=True, stop=True)
                    if mock_cc:
                        nc.sync.dma_start(d_oxout[i].ap(), d_oxin[i].ap())
                    else:
                        nc.gpsimd.collective_compute(
                            "AllReduce", mybir.AluOpType.add,
                            replica_groups=PAIRS,
                            ins=[d_oxin[i].ap()], outs=[d_oxout[i].ap()])
                    prep = pR.tile([128, 512], F32, tag="prep")
                    nc.tensor.matmul(prep[:], wm1pPt[:], otT[:],
                                     start=True, stop=True)
                    nc.scalar.copy(negrep[:, 0:512], prep[:])
                    oxs = stp.tile([16, 512], F16, tag="oxs")
                    nc.sync.dma_start(oxs[:], d_oxout[i].ap())
                    # partner half: P(oxs) - P(otT) == P(oxs - otT); doing
                    # the subtraction after the matmul keeps the fp16 sub
                    # off the critical path between the exchange and prep2
                    # bridge the prep -> negrep-copy -> first-ht latency
                    scrp = pSm.tile([16, 512], F32, tag="pzt")
                    for _ in range(8):
                        nc.tensor.matmul(scrp[:], rsnp[:, 0:16],
                                         att[:, 0:512], start=True, stop=True)
                    prep2 = pR.tile([128, 512], F32, tag="prep")
                    nc.tensor.matmul(prep2[:], wm1pPt[:], oxs[:],
                                     start=True, stop=True)
                    nc.vector.tensor_sub(negrep[:, 512:1024], prep2[:],
                                         negrep[:, 0:512])
                    # deferred S_0 writes go out here: their Pool muls and
                    # HWDGE slots must not contend with the r_t collective
                    # hops or the rt48 prep
                    flush_out()

                    # ---------- D-phase ----------
                    # ht tiles are pure engine ops (no PE): fp16
                    # relu(negrep + abias[:, j]) on DVE (4x mode) with a
                    # few on ACT; the PE only runs the 128 k-reduction
                    # matmuls.  The next step's softmax (or the S_L output)
                    # is pipelined two (sc,tb) groups behind so neither the
                    # ACT exp nor the DVE reciprocal ever stalls an engine
                    # stream in front of ht production.
                    abI = abias[:, i * NJ:(i + 1) * NJ]
                    prt_nxt = prts[i + 1] if i + 1 < NUM_STEPS else None
                    out_dram = None if i + 1 < NUM_STEPS else d_sl

                    def fin(sc):
                        sm_fin(i + 1, sc, prt_nxt, out_dram, defer=False)

                    def ht_gen(ht, src, bias, j8):
                        if j8 % 4 == 3:
                            nc.scalar.activation(ht, src, AF.Relu, bias=bias)
                        elif j8 in (5, 9, 13):
                            nc.gpsimd.tensor_scalar(ht, src, bias,
                                                    0.0, ALU.add, ALU.max)
                        else:
                            nc.vector.tensor_scalar(ht, src, bias,
                                                    0.0, ALU.add, ALU.max)

                    for tb in range(2):
                        for sc in range(NSC):
                            ps2 = pD2.tile([128, 512], F32, tag="ps2",
                                           name=f"ps2_{sc}_{tb}")
                            tsl = slice(tb * 512, (tb + 1) * 512)
                            last_tile = False
                            if not last_tile:
                                for j8 in range(16):
                                    ht = hD.tile([128, 512], F16, tag="ht",
                                                 name="ht")
                                    bias = abI[:, sc * 16 + j8:
                                               sc * 16 + j8 + 1]
                                    ht_gen(ht[:], negrep[:, tsl], bias, j8)
                                    nc.tensor.matmul(
                                        ps2[:],
                                        zbig[:, 120 - 8 * j8: 248 - 8 * j8],
                                        ht[:], start=(j8 == 0),
                                        stop=(j8 == 15))
                                # fused DVE fold+rowmax (gpsimd cannot read
                                # PSUM on trn2, so this stays on the DVE)
                                fold_max(sc, tb, ps2)
                            else:
                                # final tile: run the k-reduction per
                                # 256-column half so the first half's fold
                                # and row max hide under the second half's
                                # matmuls, shortening the output tail
                                n1 = rzt[sc]["n1"]
                                for half in range(2):
                                    cs = slice(half * 256, (half + 1) * 256)
                                    nsl = slice(tb * 512 + half * 256,
                                                tb * 512 + (half + 1) * 256)
                                    for j8 in range(16):
                                        ht2 = hD.tile([128, 256], F16,
                                                      tag="ht2", name="ht2")
                                        bias = abI[:, sc * 16 + j8:
                                                   sc * 16 + j8 + 1]
                                        ht_gen(ht2[:], negrep[:, nsl],
                                               bias, j8)
                                        nc.tensor.matmul(
                                            ps2[:, cs],
                                            zbig[:, 120 - 8 * j8:
                                                 248 - 8 * j8],
                                            ht2[:], start=(j8 == 0),
                                            stop=(j8 == 15))
                                    ssl_h = slice(sc * N + tb * 512
                                                  + half * 256,
                                                  sc * N + tb * 512
                                                  + (half + 1) * 256)
                                    nc.vector.tensor_add(
                                        shat[:, ssl_h], shat[:, ssl_h],
                                        ps2[:, cs])
                            if tb == 1:
                                if sc >= 1:
                                    sm_exp(sc - 1)
                                if sc >= 2:
                                    fin(sc - 2)
                    fin(NSC - 2)
                    sm_exp(NSC - 1)
                    fin(NSC - 1)

                flush_out()
            pAo_ctx.__exit__(None, None, None)
            pPr_ctx.__exit__(None, None, None)
            out_pool_ctx.__exit__(None, None, None)

    nc.compile()
    _BUILD_CACHE[key] = nc
    return nc


def _prep_core_inputs(core, shared, per_graph):
    """Assemble the in_map for one core."""
    b, h = core // 2, core % 2
    at_s_T, at_t_T = per_graph["ats"], per_graph["att"]
    tcols = slice(b * N + h * 512, b * N + h * 512 + 512)
    scols = slice(b * N + h * SH, b * N + h * SH + SH)
    att_cm = _chunk_major(
        np.ascontiguousarray(at_t_T[:, tcols])).astype(ml_dtypes.float8_e4m3fn)
    ats_chunks = np.ascontiguousarray(
        at_s_T[:, scols].reshape(NCHUNK, 128, SH)).astype(ml_dtypes.float8_e4m3fn)
    r_s = shared["r_s_steps"]   # [2, B, N, R]
    rs_own = np.zeros((128, NUM_STEPS * NSC * R), np.float32)
    for i in range(NUM_STEPS):
        blk = r_s[i, b, h * SH: h * SH + SH, :].reshape(NSC, 128, R)
        rs_own[:, i * NSC * R:(i + 1) * NSC * R] = (
            blk.transpose(1, 0, 2).reshape(128, NSC * R))
    return dict(
        att=att_cm,
        ats=ats_chunks,
        xshi=shared["xshi"], xslo=shared["xslo"],
        xthi=shared["xthi"], xtlo=shared["xtlo"],
        rsnp=shared["rsnp"],
        rsown=rs_own,
        w1=shared["w1"], b1c=shared["b1c"], w2=shared["w2"],
        w2st=shared["w2st"],
        b2c=shared["b2c"], wm1pA=shared["wm1pA"],
        wm1pPt=shared["wm1pPt"], bm1pc=shared["bm1pc"],
        zbig=shared["zbig"],
    )


def prepare(x_s, edge_index_s, batch_s, x_t, edge_index_t, batch_t,
            r_s_steps, W1, b1, W2, b2, Wm1, bm1, Wm2, bm2):
    """Host-side preprocessing shared by kernel() and the sim harness."""
    x_s = np.asarray(x_s, np.float32)
    x_t = np.asarray(x_t, np.float32)
    r_s_steps = np.asarray(r_s_steps, np.float32)
    W1 = np.asarray(W1, np.float32)
    b1 = np.asarray(b1, np.float32)
    W2 = np.asarray(W2, np.float32)
    b2 = np.asarray(b2, np.float32)
    Wm1 = np.asarray(Wm1, np.float32)
    bm1 = np.asarray(bm1, np.float32)
    Wm2 = np.asarray(Wm2, np.float32).reshape(R)
    bm2 = float(np.asarray(bm2, np.float32).reshape(()))

    bm2 = 0.0  # uniform shift of S_hat; softmax outputs are invariant to it
    # |Wm2|-folded MLP weights; signs live in the zbig reduction matrix
    signs = np.where(Wm2 >= 0, 1.0, -1.0)
    kp = 0  # unused; program is input-shape-only
    wm1p = Wm1 * np.abs(Wm2)[None, :]
    bm1p = bm1 * np.abs(Wm2)

    xshi, xslo = _bf_split(x_s)
    xthi, xtlo = _bf_split(x_t)
    rs_flat = r_s_steps.reshape(NUM_STEPS, NNODE, R)
    rsnp = np.zeros((128, NCHUNK, 112), np.float32)
    for i in range(NUM_STEPS):
        cm = _chunk_major(rs_flat[i]).reshape(128, NCHUNK, R)
        hi, lo = _bf_split(cm)
        rsnp[:, :, 64 * i:64 * i + 16] = hi.astype(np.float32)
        rsnp[:, :, 64 * i + 32:64 * i + 48] = lo.astype(np.float32)
    rsnp = rsnp.reshape(128, NCHUNK * 112).astype(ml_dtypes.bfloat16)

    zbig = np.zeros((128, 248), np.float32)
    for s8 in range(8):
        for k in range(R):
            zbig[s8 * R + k, 120 + s8] = signs[k]

    shared = dict(
        r_s_steps=r_s_steps,
        xshi=_chunk_major(xshi.astype(np.float32)).astype(ml_dtypes.bfloat16),
        xslo=_chunk_major(xslo.astype(np.float32)).astype(ml_dtypes.bfloat16),
        xthi=_chunk_major(xthi.astype(np.float32)).astype(ml_dtypes.bfloat16),
        xtlo=_chunk_major(xtlo.astype(np.float32)).astype(ml_dtypes.bfloat16),
        rsnp=rsnp,
        w1=W1, b1c=b1.reshape(2, 128).T.copy(),
        w2=W2, w2st=np.concatenate(
            [W2, np.zeros((16, R), np.float32), W2], 0),
        b2c=b2.reshape(R, 1),
        wm1pA=wm1p,
        wm1pPt=np.tile(-wm1p, (1, 8)).astype(np.float16),
        bm1pc=bm1p.reshape(R, 1),
        zbig=zbig.astype(np.float16),
    )
    per_graph = dict(ats=_adjT_plus_I(edge_index_s),
                     att=_adjT_plus_I(edge_index_t))
    in_maps = [_prep_core_inputs(c, shared, per_graph) for c in range(NCORES)]
    return in_maps, kp, bm2


def assemble(results):
    """Stack per-core [SH, N] outputs into full [B*N, N] S_0 / S_L."""
    s0 = np.zeros((B * N, N), np.float32)
    sl = np.zeros((B * N, N), np.float32)
    for c in range(NCORES):
        b, h = c // 2, c % 2
        rows = slice(b * N + h * SH, b * N + h * SH + SH)
        r0, rl = results[c]["s0o"], results[c]["slo"]
        if h == 1:
            # odd cores hold their own t-half in local columns 0:512
            r0 = np.concatenate([r0[:, 512:], r0[:, :512]], axis=1)
            rl = np.concatenate([rl[:, 512:], rl[:, :512]], axis=1)
        s0[rows] = r0
        sl[rows] = rl
    return s0, sl


def kernel(**inputs):
    in_maps, kp, bm2 = prepare(**inputs)
    nc = _build(kp, bm2)
    res = bass_utils.run_bass_kernel_spmd(nc, in_maps,
                                          core_ids=list(range(NCORES)))
    return assemble(res.results)

